# revision 1
# baseline (speedup 1.0000x reference)
"""Multi-head attention kernel for Trainium2, 8 NeuronCores.

Problem: B=4, S=2048, D=1024, H=16 heads, d_k=64 (fp32).
    out = softmax((Q Wq + bq)(K Wk + bk)^T / 8) (V Wv + bv) Wo + bo

Sharding: core c handles batch b = c//2 and head-group g = c%2
(8 heads, a 512-wide slice of the model dim). W_q/W_k/W_v split
column-wise, W_o row-wise; each core computes a full [2048, 1024]
partial output and the host sums core pairs and adds bo + bv@Wo
(the V-bias passes through softmax unchanged).

v3 design:
  * All matmul operands bf16 (fp32 PSUM accumulation; host pre-casts
    and pre-lays-out every DRAM tensor so each DMA is contiguous
    >=2KB-per-partition).
  * Score matmuls for the two heads of a pair issue back-to-back with
    row-half tile_positions (K=64 each) and run CONCURRENT on the PE.
  * Software-pipelined emission: scores(s+1) enters the PE queue
    before attnV(s), so ACT (exp: 256 x ~1.1us) paces the kernel.
  * attnV keeps keys-on-partitions orientation; ones column in v_aug
    makes row 64 of the accumulator the softmax denominator.
  * Normalization without any DMA: DVE reciprocal of the denominator
    row, then a 1-partition PE matmul (ones[1,64]^T @ recip[1,512])
    broadcasts it into partitions 64..127 of the SAME psum bank, and
    DVE multiplies the two halves into OT (bf16).
  * Projections and output projection are deadline-scheduled fillers
    inside the attention steps; q/k tiles rotate (bufs=2) per pair.
"""

import sys

sys.path.insert(0, '/opt/trn_rl_repo')

import numpy as np

B = 4
S = 2048
D = 1024
H = 16
DK = 64
HPC = 8          # heads per core
DH = 512         # model-dim slice per core
N_CORES = 8
NKB = S // 128   # 16 key blocks
SQ = 512

_CACHE = {}


def _build():
    import concourse.bass as bass
    import concourse.tile as tile
    from concourse import mybir
    from concourse import library_config
    import bass_rust

    # ---- workarounds for this walrus build: max ONE sync wait/instr ----
    def _patched_drain_and_barrier(self, tick_clock, wait_clock):
        drain_inst = self.nc.sync.drain()
        wait_clock.add_sem_waits(
            drain_inst.ins, tile.ScopedClock({None: tick_clock.global_clock}))
        mi = drain_inst.ins
        si = mi.sync_info
        waits = list(si.on_wait or []) if si is not None else []
        if len(waits) > 1:
            si.on_wait = waits[:1]
            for w in waits[1:]:
                d2 = self.nc.sync.drain()
                si2 = d2.ins.sync_info
                if si2 is None:
                    d2.ins.sync_info = bass_rust.SyncInfo(on_wait=[w], on_update=[])
                else:
                    si2.on_wait = [w]
        self.nc.all_engine_barrier()
        popped = self.nc._tile_sem_poison_stack.pop()
        assert popped is self._sem_poison
        self.nc.clear_and_free_semaphores(list(self.sems.allocated().values()))
        self.nc.all_engine_barrier()

    tile.TileContext._drain_and_barrier = _patched_drain_and_barrier

    def legalize_sync_waits(nc):
        for f in nc.m.functions:
            for bb in f.blocks:
                il = bb.instructions
                if not any(
                    inst.sync_info is not None
                    and len(inst.sync_info.on_wait or []) > 1
                    for inst in il
                ):
                    continue
                new = []
                for inst in il:
                    si = inst.sync_info
                    waits = list(si.on_wait or []) if si is not None else []
                    if len(waits) > 1 and inst.engine != mybir.EngineType.Unassigned:
                        eng = nc.engines[inst.engine]
                        for w in waits[:-1]:
                            nop = eng.nop()
                            nopmi = nop.ins
                            cur = nc.cur_bb.bb if hasattr(nc.cur_bb, 'bb') else nc.cur_bb
                            cil = cur.instructions
                            for k in range(len(cil) - 1, -1, -1):
                                if cil[k].name == nopmi.name:
                                    del cil[k]
                                    break
                            si2 = nopmi.sync_info
                            if si2 is None:
                                nopmi.sync_info = bass_rust.SyncInfo(
                                    on_wait=[w], on_update=[])
                            else:
                                si2.on_wait = [w]
                            new.append(nopmi)
                        si.on_wait = waits[-1:]
                    new.append(inst)
                il[:] = new

    F32 = mybir.dt.float32
    F32R = mybir.dt.float32r
    BF16 = mybir.dt.bfloat16
    nc = bass.Bass('TRN2', target_bir_lowering=False, debug=False)

    # host-prepped layouts: every tensor matches its SBUF tile layout so
    # DMAs are fully contiguous per partition.
    xq4 = nc.dram_tensor('xq4', [4, 128, 8, SQ], BF16, kind='ExternalInput').ap()
    xk4 = nc.dram_tensor('xk4', [4, 128, 8, SQ], BF16, kind='ExternalInput').ap()
    xv16 = nc.dram_tensor('xv16', [16, 128, 8, 128], BF16,
                          kind='ExternalInput').ap()
    wq4 = nc.dram_tensor('wq4', [4, 128, 8, 128], BF16,
                         kind='ExternalInput').ap()
    wk4 = nc.dram_tensor('wk4', [4, 128, 8, 128], BF16,
                         kind='ExternalInput').ap()
    wv8 = nc.dram_tensor('wv8', [128, 8, DH], BF16, kind='ExternalInput').ap()
    wo4 = nc.dram_tensor('wo4', [128, 4, D], BF16, kind='ExternalInput').ap()
    bq = nc.dram_tensor('bq', [128, 4], F32, kind='ExternalInput').ap()
    bk = nc.dram_tensor('bk', [128, 4], F32, kind='ExternalInput').ap()
    out = nc.dram_tensor('out', [S, 4 * D], BF16, kind='ExternalOutput').ap()

    EXP = mybir.ActivationFunctionType.Exp

    from contextlib import ExitStack
    with tile.TileContext(nc) as tc:
        with ExitStack() as _es:
            _p = lambda *a, **k: _es.enter_context(tc.tile_pool(*a, **k))
            constp = _p(name='const', bufs=1)
            qtp = _p(name='qtp', bufs=2)
            ktp = _p(name='ktp', bufs=2)
            otv = _p(name='otv', bufs=1)
            wts = _p(name='wts', bufs=1)
            xqp = _p(name='xq', bufs=1)
            xkp = _p(name='xk', bufs=1)
            xvp = _p(name='xv', bufs=1)
            atp = _p(name='atp', bufs=6)
            rrp = _p(name='rrp', bufs=2)
            bcp = _p(name='bcp', bufs=2)
            ocp = _p(name='ocp', bufs=4)
            drp = _p(name='drp', bufs=1, space='DRAM')
            obuf = _p(name='obuf', bufs=3)
            pwp = _p(name='pw', bufs=2, space='PSUM')
            accp = _p(name='acc', bufs=2, space='PSUM')
            projp = _p(name='prj', bufs=1, space='PSUM')
            outp = _p(name='op', bufs=1, space='PSUM')

            bq_t = constp.tile([128, 4], F32, name='bq_t')
            bk_t = constp.tile([128, 4], F32, name='bk_t')
            warm = constp.tile([1, 2], F32, name='warm')
            nc.sync.dma_start(bq_t[:], bq[:])
            nc.sync.dma_start(bk_t[:], bk[:])
            scratch = drp.tile([32, SQ], F32, name='scratch')
            nc.vector.memset(warm[:], 0.0)
            # load the exp table set early (one-time ~2.7us)
            nc.scalar.activation(warm[0:1, 0:1], warm[0:1, 1:2], EXP)

            # persistent / rotating activation tiles (bf16)
            OT = [otv.tile([128, S], BF16, name=f'OT{j}', tag=f'OT{j}')
                  for j in range(4)]
            v_aug = otv.tile([128, HPC * NKB * 65], BF16, name='v_aug',
                             tag='v_aug')
            v_view = v_aug.rearrange('p (h c w) -> p h c w', h=HPC, c=NKB)
            # only the ones-columns need initializing (128 strided cols,
            # ~0.2us, vs 7us for the whole tile)
            nc.vector.memset(v_view[:, :, :, 64:65], 1.0)
            qt_tiles = {}
            kt_tiles = {}

            # ---- weights + x staged in SBUF (bf16) ----
            wq_t = wts.tile([128, 4, 8, 128], BF16, name='wq_t')
            wk_t = wts.tile([128, 4, 8, 128], BF16, name='wk_t')
            wv_t = wts.tile([128, 8, DH], BF16, name='wv_t')
            wo_t = wts.tile([128, 4, D], BF16, name='wo_t')
            xq_sc = [xqp.tile([128, 8, SQ], BF16, name=f'xq{sc}',
                              tag=f'xq{sc}') for sc in range(4)]
            xk_sc = [xkp.tile([128, 8, SQ], BF16, name=f'xk{sc}',
                              tag=f'xk{sc}') for sc in range(4)]
            xv_tb = [xvp.tile([128, 8, 128], BF16, name=f'xv{tb}',
                              tag=f'xv{tb}') for tb in range(16)]

            # Input DMAs: the ACT hardware queue carries ONLY what the first
            # exps need (its triggers sit in front of the exp stream), the
            # SP queue everything else in first-need order.
            nc.scalar.dma_start(xk_sc[0][:], xk4[0])
            nc.scalar.dma_start(wv_t[:], wv8[:])
            for tb in range(6):
                nc.scalar.dma_start(xv_tb[tb][:], xv16[tb])
            nc.scalar.dma_start(xk_sc[1][:], xk4[1])
            nc.sync.dma_start(wq_t[:, 0], wq4[0])
            nc.sync.dma_start(wk_t[:, 0], wk4[0])
            nc.sync.dma_start(xq_sc[0][:], xq4[0])
            for tb in range(6, 10):
                nc.sync.dma_start(xv_tb[tb][:], xv16[tb])
            nc.sync.dma_start(xk_sc[2][:], xk4[2])
            for tb in range(10, 16):
                nc.sync.dma_start(xv_tb[tb][:], xv16[tb])
            nc.sync.dma_start(xk_sc[3][:], xk4[3])
            for sc in range(1, 4):
                nc.sync.dma_start(xq_sc[sc][:], xq4[sc])
            nc.sync.dma_start(wq_t[:, 1], wq4[1])
            nc.sync.dma_start(wk_t[:, 1], wk4[1])
            nc.sync.dma_start(wo_t[:], wo4[:])
            for j in range(2, 4):
                nc.sync.dma_start(wq_t[:, j], wq4[j])
                nc.sync.dma_start(wk_t[:, j], wk4[j])

            # ---------------- filler machinery ----------------
            # (deadline, earliest, seq, cost, fn): mandatory before step
            # `deadline` (emission order = dependency order); budget pumping
            # won't pull an item before step `earliest`.
            fillers = []
            state = {'debt': 0.0, 'seq': 0}

            import bisect

            def add_filler(deadline, earliest, cost_fn):
                cost, fn = cost_fn
                bisect.insort(
                    fillers, (deadline, state['seq'], earliest, cost, fn))
                state['seq'] += 1

            def pump(step_idx, budget_ns):
                while fillers and fillers[0][0] <= step_idx:
                    _, _, _, cost, fn = fillers.pop(0)
                    fn()
                    state['debt'] -= cost
                state['debt'] = min(max(state['debt'] + budget_ns, -2000),
                                    900)
                # hold budget pumping just before qc boundaries so the DVE
                # FIFO drains ahead of the norm/acc-release chain
                if step_idx % 16 >= 14:
                    return
                while (fillers and state['debt'] > 0
                       and fillers[0][2] <= step_idx):
                    _, _, _, cost, fn = fillers.pop(0)
                    fn()
                    state['debt'] -= cost

            def proj_chunk(which, j, sc, half):
                # 4 accumulating matmuls; second half adds bias into qT/kT
                def fn():
                    tiles = qt_tiles if which == 'q' else kt_tiles
                    pool = qtp if which == 'q' else ktp
                    if j not in tiles:
                        tiles[j] = pool.tile([128, S], BF16,
                                             name=f'{which}T', tag='t')
                    key = ('pa', which, j, sc)
                    if half == 0:
                        acc = projp.tile([128, SQ], F32, name='pacc',
                                         tag='fil')
                        state[key] = acc
                    else:
                        acc = state.pop(key)
                    w_t = wq_t if which == 'q' else wk_t
                    x_t = (xq_sc if which == 'q' else xk_sc)[sc]
                    for kc in range(4 * half, 4 * half + 4):
                        nc.tensor.matmul(
                            acc[:],
                            w_t[:, j, kc, :],
                            x_t[:, kc, :],
                            start=(kc == 0), stop=(kc == 7))
                    if half == 1:
                        bias = bq_t if which == 'q' else bk_t
                        nc.vector.tensor_scalar_add(
                            tiles[j][:, sc * SQ:(sc + 1) * SQ], acc[:],
                            bias[:, j:j + 1])
                return (450, fn)

            def v_chunk(tb, half):
                def fn():
                    key = ('va', tb)
                    if half == 0:
                        acc = projp.tile([128, DH], F32, name='pacc',
                                         tag='fil')
                        state[key] = acc
                    else:
                        acc = state.pop(key)
                    for kc in range(4 * half, 4 * half + 4):
                        nc.tensor.matmul(
                            acc[:],
                            xv_tb[tb][:, kc, :],
                            wv_t[:, kc, :],
                            start=(kc == 0), stop=(kc == 7))
                    if half == 1:
                        nc.vector.tensor_copy(
                            v_view[:, :, tb, 0:64],
                            acc.rearrange('p (h d) -> p h d', h=HPC))
                return (450, fn)

            def outproj_item(j, tb, half):
                def fn():
                    # in the last window the proj/acc psum banks are idle:
                    # rotate over them so the MM -> copy -> DMA chain
                    # pipelines wider and the tail drains fast.
                    if j == 3 and tb >= 12:
                        pool = (outp, projp, accp)[(tb + half) % 3]
                        tag = 'acc' if pool is accp else 'fil'
                    elif j == 3:
                        pool = (outp, projp)[(tb + half) % 2]
                        tag = 'fil'
                    else:
                        pool, tag = outp, 'fil'
                    o = pool.tile([128, SQ], F32, name='oacc', tag=tag)
                    nc.tensor.matmul(
                        o[:],
                        OT[j][:, tb * 128:(tb + 1) * 128],
                        wo_t[:, j, half * SQ:(half + 1) * SQ],
                        start=True, stop=True)
                    ob = obuf.tile([128, SQ], BF16, name='ob', tag='ob')
                    nc.vector.tensor_copy(ob[:], o[:])
                    nc.sync.dma_start(
                        out[tb * 128:(tb + 1) * 128,
                            j * D + half * SQ:j * D + (half + 1) * SQ],
                        ob[:])
                return (300, fn)

            # j0 q/k projection for the first query/key chunk (head start)
            for which in ('q', 'k'):
                for half in range(2):
                    proj_chunk(which, 0, 0, half)[1]()

            for tb in range(16):
                add_filler(tb, 0, v_chunk(tb, 0))
                add_filler(tb, 0, v_chunk(tb, 1))
            for sc in range(1, 4):
                for half in range(2):
                    add_filler(4 * sc - 3, 0, proj_chunk('k', 0, sc, half))
                    add_filler(16 * sc - 10, 0, proj_chunk('q', 0, sc, half))
            # pairs j>=1: spread their projection uniformly over window j-1
            for j in range(1, 4):
                est = 64 * (j - 1) + (18 if j == 1 else 0)
                idx = 0
                for sc in range(4):
                    for which in ('k', 'q'):
                        hard = 64 * j + (4 if which == 'k' else 16) * sc - 1
                        for half in range(2):
                            spread = 64 * (j - 1) + 16 + 3 * idx
                            idx += 1
                            add_filler(min(hard, spread), est,
                                       proj_chunk(which, j, sc, half))
            # ---------------- attention steps (software pipelined) -------
            steps = [(j, qc, kb)
                     for j in range(4) for qc in range(4) for kb in range(16)]
            accs = {}
            prev = None

            def emit_attnv(pj, pqc, pkb, at):
                if pkb == 0:
                    accs[(pj, pqc)] = (
                        accp.tile([128, SQ], F32, name='accA', tag='acc'),
                        accp.tile([128, SQ], F32, name='accB', tag='acc'))
                accA, accB = accs[(pj, pqc)]
                for hi, acc in ((0, accA), (1, accB)):
                    h = 2 * pj + hi
                    nc.tensor.matmul(
                        acc[0:65, :],
                        v_view[:, h, pkb, 0:65],
                        at[:, hi * SQ:(hi + 1) * SQ],
                        start=(pkb == 0), stop=(pkb == NKB - 1))
                if pkb == NKB - 1:
                    accA, accB = accs.pop((pj, pqc))
                    cur = (pj * 4 + pqc) * 16 + 15
                    late = cur > 235
                    ocs = []
                    for hi, acc in ((0, accA), (1, accB)):
                        # copy to SBUF right away: frees the psum bank fast;
                        # everything downstream runs deferred so it never
                        # blocks the PE/DVE FIFOs.
                        oc = ocp.tile([65, SQ], F32, name='oc', tag='oc')
                        nc.vector.tensor_copy(oc[:], acc[0:65, :])
                        ocs.append(oc)
                    box = {}

                    def denom_fn(ocA=ocs[0], ocB=ocs[1], pj=pj, pqc=pqc,
                                 box=box):
                        # denominator rows to partitions 0/32 of one tile
                        # (sbuf-to-sbuf DMAs; DVE writes must be 32-aligned)
                        # so ONE exact reciprocal (3.3us) serves both heads,
                        # then the DRAM round-trip partition broadcast.
                        dd = rrp.tile([33, SQ], F32, name='dd', tag='rr')
                        nc.sync.dma_start(dd[0:1, :], ocA[64:65, :])
                        nc.sync.dma_start(dd[32:33, :], ocB[64:65, :])
                        nc.vector.reciprocal(dd[:], dd[:])
                        r0 = 8 * pj + 4 * (pqc % 2)
                        for k, row in ((0, 0), (1, 32)):
                            nc.sync.dma_start(scratch[r0 + k:r0 + k + 1, :],
                                              dd[row:row + 1, :])
                            bcs = bcp.tile([64, SQ], F32, name='bcs',
                                           tag='bc')
                            nc.sync.dma_start(
                                bcs[:],
                                scratch[r0 + k:r0 + k + 1, :]
                                .partition_broadcast(64))
                            box[k] = bcs
                    # j3 windows drain eagerly (they feed the kernel tail);
                    # emission order must stay denom -> mul -> outproj.
                    if late:
                        d_den, d_mul, d_op, d_stride = 1, 2, 3, 1
                    elif pj == 3:
                        d_den, d_mul, d_op, d_stride = 1, 4, 5, 1
                    else:
                        d_den, d_mul, d_op, d_stride = 3, 12, 14, 2
                    add_filler(cur + d_den, 0, (100, denom_fn))

                    for hi in range(2):
                        def mul_fn(hi=hi, oc=ocs[hi], pj=pj, pqc=pqc,
                                   box=box):
                            nc.vector.tensor_mul(
                                OT[pj][hi * 64:(hi + 1) * 64,
                                       pqc * SQ:(pqc + 1) * SQ],
                                oc[0:64, :], box[hi][:])
                        add_filler(cur + d_mul, 0, (100, mul_fn))
                    k = 0
                    for tb in range(4 * pqc, 4 * pqc + 4):
                        for half in range(2):
                            add_filler(cur + d_op + d_stride * k, 0,
                                       outproj_item(pj, tb, half))
                            k += 1

            for i, step in enumerate(steps):
                j, qc, kb = step
                pw = pwp.tile([128, 2 * SQ], F32, name='pw', tag='pw')
                for hi in range(2):
                    po = hi * 64
                    nc.tensor.matmul(
                        pw[:, hi * SQ:(hi + 1) * SQ],
                        kt_tiles[j][po:po + 64, kb * 128:(kb + 1) * 128],
                        qt_tiles[j][po:po + 64, qc * SQ:(qc + 1) * SQ],
                        start=True, stop=True)
                at = atp.tile([128, 2 * SQ], BF16, name='at', tag='at')
                nc.scalar.activation(at[:], pw[:], EXP)
                if prev is not None:
                    emit_attnv(*prev)
                prev = (j, qc, kb, at)
                pump(i, 900)
            emit_attnv(*prev)
            while fillers:
                _, _, _, cost, fn = fillers.pop(0)
                fn()

    legalize_sync_waits(nc)
    return nc


def _get_nc():
    if 'nc' not in _CACHE:
        _CACHE['nc'] = _build()
    return _CACHE['nc']


def _make_in_maps(Q, K, V, Wq, bq, Wk, bk, Wv, bv, Wo):
    import ml_dtypes
    f32 = np.float32
    bf16 = ml_dtypes.bfloat16
    Q = np.asarray(Q, f32)
    K = np.asarray(K, f32)
    V = np.asarray(V, f32)
    Wq = np.asarray(Wq, f32)
    Wk = np.asarray(Wk, f32)
    Wv = np.asarray(Wv, f32)
    Wo = np.asarray(Wo, f32)
    bq = np.asarray(bq, f32)
    bk = np.asarray(bk, f32)
    scale = f32(1.0 / np.sqrt(DK))

    def xlayout(x_t, inner):
        # x_t [1024, 2048] -> [2048//inner, 128, 8, inner]
        return np.ascontiguousarray(
            x_t.reshape(8, 128, S // inner, inner).transpose(2, 1, 0, 3)
        ).astype(bf16)

    def wlayout(w):
        # w [1024, 512] -> [4, 128, 8, 128]  (pair-major)
        return np.ascontiguousarray(
            w.reshape(8, 128, 4, 128).transpose(2, 1, 0, 3)).astype(bf16)

    in_maps = []
    for c in range(N_CORES):
        b, g = c // 2, c % 2
        cs = slice(g * DH, (g + 1) * DH)
        wv_s = Wv[:, cs]
        in_maps.append({
            'xq4': xlayout(Q[b].T, SQ),
            'xk4': xlayout(K[b].T, SQ),
            'xv16': xlayout(V[b].T, 128),
            'wq4': wlayout(Wq[:, cs] * scale),
            'wk4': wlayout(Wk[:, cs]),
            'wv8': np.ascontiguousarray(
                wv_s.reshape(8, 128, DH).transpose(1, 0, 2)).astype(bf16),
            'wo4': np.ascontiguousarray(
                Wo[cs, :].reshape(4, 128, D).transpose(1, 0, 2)).astype(bf16),
            'bq': np.ascontiguousarray((bq[cs] * scale).reshape(4, 128).T),
            'bk': np.ascontiguousarray(bk[cs].reshape(4, 128).T),
        })
    return in_maps


def _patch_ldw_opt():
    # walrus ships with LDWEIGHTS dedup disabled; consecutive matmuls
    # sharing a stationary operand then reload it every time.
    from concourse import bass_utils
    if getattr(bass_utils, '_ldw_opt_patched', False):
        return
    orig = bass_utils.run_command

    def run_command2(argv, **kw):
        argv = ['--enable-ldw-opt=true' if a == '--enable-ldw-opt=false'
                else a for a in argv]
        return orig(argv, **kw)

    bass_utils.run_command = run_command2
    bass_utils._ldw_opt_patched = True


def _run(in_maps, trace=False, tmpdir=None):
    from concourse import bass_utils
    nc = _get_nc()
    kw = {}
    if trace:
        kw = dict(trace=True, tmpdir=tmpdir)
    return bass_utils.run_bass_kernel_spmd(
        nc, in_maps, core_ids=list(range(N_CORES)), **kw)


def kernel(Q, K, V, Wq, bq, Wk, bk, Wv, bv, Wo, bo):
    in_maps = _make_in_maps(Q, K, V, Wq, bq, Wk, bk, Wv, bv, Wo)
    res = _run(in_maps)
    # V-bias passes through softmax (attention rows sum to 1), so its
    # contribution is the constant row bv @ Wo, added here exactly.
    const_row = (np.asarray(bv, np.float64) @ np.asarray(Wo, np.float64)
                 + np.asarray(bo, np.float64)).astype(np.float32)
    outs = [np.asarray(r['out'], np.float32).reshape(S, 4, D).sum(axis=1)
            for r in res.results]
    full = np.stack(
        [outs[2 * b] + outs[2 * b + 1] + const_row[None, :]
         for b in range(B)], axis=0)
    return full.astype(np.float32)

